# revision 17
# baseline (speedup 1.0000x reference)
"""NefClass fuzzy-rule classifier kernel for 8x Trainium2 NeuronCores.

Pipeline (per core, batch-sharded 8 ways):
  1. Replicate x [16, 2048] -> x_rep [112, 2048] (one row per (feature, MF)).
  2. Memberships: mem = min((x-a)/(b-a), (c-x)/(c-b)) as two per-partition
     affine maps (ACT) + a DVE min. ReLU/clip deferred: clip-at-1 is a no-op
     for triangular MFs (min(left,right) <= 1 always) and relu commutes with
     min, so it is applied once to the final per-rule firing.
  3. Per-rule gather: one-hot matmul on PE. For rule-tile t (128 rules) and
     feature f, psum = G[f,t].T @ mem[f]  gathers mem[f, cond[r,f], :] into
     rule order. Min-accumulated into firing[t] by ACT(copy)+DVE(min).
  4. Class segment-sum: one-hot class matmul accumulating [10, 2048] in PSUM.
  5. Output written as [10, 2048] per core; host transposes/concats.

The rule tables and MF params reach the device as runtime inputs (one-hot
matrices built on host), so the compiled program is input-independent and
cached across calls.
"""

import numpy as np
import ml_dtypes

import concourse.bass as bass
import concourse.mybir as mybir
import concourse.tile as tile
from concourse.bass_utils import run_bass_kernel_spmd

F = 16          # features
M = 7           # membership functions per feature
C = 10          # classes
R = 512         # rules
B = 16384       # batch
NCORES = 8
BL = B // NCORES     # 2048 batch per core
FM = F * M           # 112
RT = R // 128        # 4 rule tiles of 128 rules
NCH = BL // 512      # 4 psum free-dim chunks of 512

F32 = mybir.dt.float32
BF16 = mybir.dt.bfloat16
BF16_NP = ml_dtypes.bfloat16

AF = mybir.ActivationFunctionType
ALU = mybir.AluOpType

_PROGRAM = None


def _split_multi_waits(nc):
    """This container's walrus codegen only encodes ONE sem wait per
    instruction. Hoist extra waits into standalone NOPs on the same engine
    immediately before the instruction (same semantics: the engine's
    sequencer stalls at the NOP)."""
    k = 0
    for fn in nc.m.functions:
        for blk in fn.blocks:
            old = list(blk.instructions)
            new = []
            changed = False
            for ins in old:
                si = getattr(ins, "sync_info", None)
                eng = getattr(ins, "engine", None)
                if si is not None and len(si.on_wait) > 1 and eng is not None:
                    waits = list(si.on_wait)
                    for w in waits[:-1]:
                        nop = mybir.InstNoOp(
                            name=f"{ins.name}_ws{k}",
                            sync_info=mybir.SyncInfo(on_wait=[w], on_update=[]),
                            bass_nofuse=True,
                            engine=eng,
                        )
                        k += 1
                        new.append(nop)
                    ins.sync_info = mybir.SyncInfo(
                        on_wait=[waits[-1]], on_update=list(si.on_update)
                    )
                    changed = True
                new.append(ins)
            if changed:
                blk.instructions = new


def _build_program():
    nc = bass.Bass("TRN2", target_bir_lowering=False)

    x_d = nc.dram_tensor("x", [F, BL], F32, kind="ExternalInput").ap()
    prm_d = nc.dram_tensor("prm", [FM, 4], F32, kind="ExternalInput").ap()
    gh_d = nc.dram_tensor("gh", [FM, F * RT * 128], BF16, kind="ExternalInput").ap()
    ch_d = nc.dram_tensor("ch", [128, RT * C], BF16, kind="ExternalInput").ap()
    out_d = nc.dram_tensor("out", [C, BL], F32, kind="ExternalOutput").ap()

    with tile.TileContext(nc) as tc:
        with (
            tc.tile_pool(name="const", bufs=1) as constp,
            tc.tile_pool(name="work", bufs=1) as workp,
            tc.tile_pool(name="fire", bufs=1) as firep,
            tc.tile_pool(name="psum", bufs=2, space="PSUM") as psump,
            tc.tile_pool(name="psumc", bufs=1, space="PSUM") as psumcp,
        ):
            prm = constp.tile([FM, 4], F32)
            nc.sync.dma_start(prm[:], prm_d[:])
            gh = constp.tile([FM, F * RT * 128], BF16)
            nc.sync.dma_start(gh[:], gh_d[:])
            ch = constp.tile([128, RT * C], BF16)
            nc.sync.dma_start(ch[:], ch_d[:])

            # x replicated across the 7 MF rows of each feature (step-0
            # broadcast DMAs don't replicate on this DMA engine — use 7
            # strided-dest DMAs; the wait-split pass handles the sem fan-in)
            xr = workp.tile([FM, BL], F32)
            xr3 = xr[:].rearrange("(f m) b -> f m b", m=M)
            for m in range(M):
                nc.sync.dma_start(xr3[:, m, :], x_d[:, :])

            # fences: pull prm and xr through DVE first — compute instrs can
            # encode only one sem wait, and same-sem DVE waits merge, so the
            # affines below end up with at most one wait condition
            prm_s = workp.tile([FM, 4], F32)
            nc.vector.tensor_copy(out=prm_s[:], in_=prm[:])
            xtouch = workp.tile([FM, 1], F32)
            nc.vector.tensor_copy(out=xtouch[:], in_=xr[:, 0:1])

            # memberships (raw: relu deferred to firing)
            left = workp.tile([FM, BL], F32)
            right = workp.tile([FM, BL], F32)
            nc.vector.tensor_scalar(
                out=left[:], in0=xr[:], scalar1=prm_s[:, 0:1], scalar2=prm_s[:, 1:2],
                op0=ALU.mult, op1=ALU.add,
            )
            nc.vector.tensor_scalar(
                out=right[:], in0=xr[:], scalar1=prm_s[:, 2:3], scalar2=prm_s[:, 3:4],
                op0=ALU.mult, op1=ALU.add,
            )
            mem = workp.tile([FM, BL], BF16)
            nc.vector.tensor_tensor(
                out=mem[:], in0=left[:], in1=right[:], op=ALU.min
            )

            # per-rule gather + min over features
            firing = []
            HB = BL // 2  # 1024-wide psum chunks: 2 banks each, 2 bufs = 4 banks
            for t in range(RT):
                fir = firep.tile([128, BL], F32, tag=f"fir{t}")
                for f in range(F):
                    lhsT = gh[:, (f * RT + t) * 128 : (f * RT + t + 1) * 128]
                    for h in range(2):
                        ps = psump.tile([128, HB], F32, tag="gather")
                        for n in range(2):
                            nc.tensor.matmul(
                                out=ps[:, 512 * n : 512 * (n + 1)],
                                lhsT=lhsT,
                                rhs=mem[:, h * HB + 512 * n : h * HB + 512 * (n + 1)],
                                start=True,
                                stop=True,
                            )
                        fslice = fir[:, h * HB : (h + 1) * HB]
                        if f == 0:
                            nc.vector.tensor_copy(out=fslice, in_=ps[:])
                        else:
                            nc.vector.tensor_tensor(
                                out=fslice, in0=fslice, in1=ps[:], op=ALU.min
                            )
                fir_r = firep.tile([128, BL], BF16, tag=f"fir_r{t}")
                nc.vector.tensor_scalar(
                    out=fir_r[:], in0=fir[:], scalar1=0.0, scalar2=None, op0=ALU.max
                )
                firing.append(fir_r)

            # class segment-sum via one-hot matmul, accumulated over rule tiles
            psc = psumcp.tile([C, BL], F32)
            for t in range(RT):
                for n in range(NCH):
                    nc.tensor.matmul(
                        out=psc[:, 512 * n : 512 * (n + 1)],
                        lhsT=ch[:, t * C : (t + 1) * C],
                        rhs=firing[t][:, 512 * n : 512 * (n + 1)],
                        start=(t == 0),
                        stop=(t == RT - 1),
                    )
            outs = workp.tile([C, BL], F32)
            nc.vector.tensor_copy(out=outs[:], in_=psc[:])
            nc.sync.dma_start(out_d[:], outs[:])

    _split_multi_waits(nc)
    return nc


def _host_inputs(x, mf_abc, rule_conditions, rule_classes):
    x = np.ascontiguousarray(np.asarray(x, dtype=np.float32))
    abc = np.asarray(mf_abc, dtype=np.float32).reshape(FM, 3)
    cond = np.asarray(rule_conditions).astype(np.int64)
    cls = np.asarray(rule_classes).astype(np.int64)

    a, b_, c_ = abc[:, 0], abc[:, 1], abc[:, 2]
    w1 = 1.0 / (b_ - a)
    p2 = -1.0 / (c_ - b_)
    prm = np.stack([w1, -a * w1, p2, -c_ * p2], axis=1).astype(np.float32)

    j = np.arange(R)
    t_idx, jj = j // 128, j % 128
    gh = np.zeros([FM, F, RT, 128], dtype=BF16_NP)
    for f in range(F):
        gh[f * M + cond[:, f], f, t_idx, jj] = 1
    gh = np.ascontiguousarray(gh.reshape(FM, F * RT * 128))

    ch = np.zeros([128, RT, C], dtype=BF16_NP)
    ch[jj, t_idx, cls] = 1
    ch = np.ascontiguousarray(ch.reshape(128, RT * C))

    return x, prm, gh, ch


def kernel(x, mf_abc, rule_conditions, rule_classes):
    global _PROGRAM
    if _PROGRAM is None:
        _PROGRAM = _build_program()

    xf, prm, gh, ch = _host_inputs(x, mf_abc, rule_conditions, rule_classes)

    in_maps = [
        {
            "x": np.ascontiguousarray(xf[:, i * BL : (i + 1) * BL]),
            "prm": prm,
            "gh": gh,
            "ch": ch,
        }
        for i in range(NCORES)
    ]
    res = run_bass_kernel_spmd(_PROGRAM, in_maps, core_ids=list(range(NCORES)))
    out = np.concatenate([r["out"].T for r in res.results], axis=0)
    return np.ascontiguousarray(out.astype(np.float32))


# revision 23
# speedup vs baseline: 1.5605x; 1.5605x over previous
"""NefClass fuzzy-rule classifier kernel for 8x Trainium2 NeuronCores.

Math: out[b,c] = sum_{r: class[r]=c} relu(min_f raw_mem[f, cond[r,f], b])
where raw_mem = min((x-a)/(b-a), (c-x)/(c-b)) (relu commutes with min, and
min(left,right) <= 1 always for triangular MFs, so clip reduces to one relu
applied to the final firing).

Per core (batch-sharded 8 ways, 2048 cols each):
  1. x -> x_rep [112, 2048]; raw memberships via ACT affines + DVE min (bf16).
  2. Pair tables: for each pair of features (2g, 2g+1), a 49-row table of
     min(mem_f1[m1], mem_f2[m2]) for all (m1, m2) combos. Built by PE
     replication matmuls (one-hot lhsT) + ACT copy + DVE min. Two groups are
     packed per [128, B] tile at partition bases 0 and 64 (matmul rhs base
     partition must be 0/32/64).
  3. Rule firing: for each rule tile (128 rules), gather one 49-combo row per
     pair group via one-hot PE matmul, then min-combine the 8 group values:
     ACT copies half to SBUF bf16 (DVE tree mins at 2x), DVE chains the rest
     straight from PSUM. Final relu on DVE.
  4. Class segment-sum via one-hot class matmul accumulating [10, B] in PSUM.
  5. Output [10, 2048] per core; host transposes/concats.

Rule tables and MF params are runtime inputs (host-built one-hot matrices),
so the compiled program is input-independent and cached.
"""

import numpy as np
import ml_dtypes

import concourse.bass as bass
import concourse.mybir as mybir
import concourse.tile as tile
from concourse.bass_utils import run_bass_kernel_spmd

F = 16          # features
M = 7           # membership functions per feature
C = 10          # classes
R = 512         # rules
B = 16384       # batch
NCORES = 8
BL = B // NCORES     # 2048 batch per core
FM = F * M           # 112
RT = R // 128        # 4 rule tiles of 128 rules
G = F // 2           # 8 pair groups
NP = G // 2          # 4 packed table tiles (2 groups per tile)
MM2 = M * M          # 49 combos per pair
HB = 1024            # psum chunk width in rule phase
NH = BL // HB        # 2 chunks
N_ACT_COPY = 4       # groups (of 8) drained via ACT copies per chunk

F32 = mybir.dt.float32
BF16 = mybir.dt.bfloat16
BF16_NP = ml_dtypes.bfloat16

AF = mybir.ActivationFunctionType
ALU = mybir.AluOpType

_PROGRAM = None


def _split_multi_waits(nc):
    """This container's walrus codegen only encodes ONE sem wait per
    instruction. Hoist extra waits into standalone NOPs on the same engine
    immediately before the instruction (same semantics: the engine's
    sequencer stalls at the NOP)."""
    k = 0
    for fn in nc.m.functions:
        for blk in fn.blocks:
            old = list(blk.instructions)
            new = []
            changed = False
            for ins in old:
                si = getattr(ins, "sync_info", None)
                eng = getattr(ins, "engine", None)
                if si is not None and len(si.on_wait) > 1 and eng is not None:
                    waits = list(si.on_wait)
                    for w in waits[:-1]:
                        nop = mybir.InstNoOp(
                            name=f"{ins.name}_ws{k}",
                            sync_info=mybir.SyncInfo(on_wait=[w], on_update=[]),
                            bass_nofuse=True,
                            engine=eng,
                        )
                        k += 1
                        new.append(nop)
                    ins.sync_info = mybir.SyncInfo(
                        on_wait=[waits[-1]], on_update=list(si.on_update)
                    )
                    changed = True
                new.append(ins)
            if changed:
                blk.instructions = new


def _build_program():
    nc = bass.Bass("TRN2", target_bir_lowering=False)

    x_d = nc.dram_tensor("x", [F, BL], F32, kind="ExternalInput").ap()
    prm_d = nc.dram_tensor("prm", [FM, 4], F32, kind="ExternalInput").ap()
    # replication one-hots: L and R side, 4 packed tiles each, [112, 128]
    rl_d = nc.dram_tensor("rl", [FM, NP * 128], BF16, kind="ExternalInput").ap()
    rr_d = nc.dram_tensor("rr", [FM, NP * 128], BF16, kind="ExternalInput").ap()
    # pair-combo gather one-hots; odd groups live at partition base 64 to
    # match their rhs table half (matmul needs equal base partitions)
    gp_d = nc.dram_tensor("gp", [128, G * RT * 128], BF16, kind="ExternalInput").ap()
    ch_d = nc.dram_tensor("ch", [128, RT * C], BF16, kind="ExternalInput").ap()
    out_d = nc.dram_tensor("out", [C, BL], F32, kind="ExternalOutput").ap()

    with tile.TileContext(nc) as tc:
        with (
            tc.tile_pool(name="const", bufs=1) as constp,
            tc.tile_pool(name="work", bufs=1) as workp,
            tc.tile_pool(name="tab", bufs=1) as tabp,
            tc.tile_pool(name="fire", bufs=1) as firep,
            tc.tile_pool(name="cpy", bufs=2) as cpp,
        ):
            prm = constp.tile([FM, 4], F32)
            nc.sync.dma_start(prm[:], prm_d[:])
            rl = constp.tile([FM, NP * 128], BF16)
            nc.sync.dma_start(rl[:], rl_d[:])
            rr = constp.tile([FM, NP * 128], BF16)
            nc.sync.dma_start(rr[:], rr_d[:])
            gp = constp.tile([128, G * RT * 128], BF16)
            nc.sync.dma_start(gp[:], gp_d[:])
            ch = constp.tile([128, RT * C], BF16)
            nc.sync.dma_start(ch[:], ch_d[:])

            # x replicated across the 7 MF rows of each feature
            xr = workp.tile([FM, BL], F32)
            xr3 = xr[:].rearrange("(f m) b -> f m b", m=M)
            for m in range(M):
                nc.sync.dma_start(xr3[:, m, :], x_d[:, :])

            # raw memberships (relu deferred to firing)
            left = workp.tile([FM, BL], F32)
            right = workp.tile([FM, BL], F32)
            nc.scalar.activation(
                left[:], xr[:], AF.Identity, scale=prm[:, 0:1], bias=prm[:, 1:2]
            )
            nc.scalar.activation(
                right[:], xr[:], AF.Identity, scale=prm[:, 2:3], bias=prm[:, 3:4]
            )
            mem = workp.tile([FM, BL], BF16)
            nc.vector.tensor_tensor(
                out=mem[:], in0=left[:], in1=right[:], op=ALU.min
            )

            # ---- pair tables: T_p holds group 2p at partitions 0..48 and
            # group 2p+1 at partitions 64..112 ----
            tables = []
            with (
                tc.tile_pool(name="psl", bufs=1, space="PSUM") as pslp,
                tc.tile_pool(name="psr", bufs=1, space="PSUM") as psrp,
            ):
                for p in range(NP):
                    psl = pslp.tile([128, BL], F32, tag="psl")
                    psr = psrp.tile([128, BL], F32, tag="psr")
                    for n in range(BL // 512):
                        sl = slice(512 * n, 512 * (n + 1))
                        nc.tensor.matmul(
                            out=psl[:, sl], lhsT=rl[:, 128 * p : 128 * (p + 1)],
                            rhs=mem[:, sl], start=True, stop=True,
                        )
                        nc.tensor.matmul(
                            out=psr[:, sl], lhsT=rr[:, 128 * p : 128 * (p + 1)],
                            rhs=mem[:, sl], start=True, stop=True,
                        )
                    cl = workp.tile([128, BL], BF16, tag="cl")
                    nc.scalar.activation(cl[:], psl[:], AF.Copy)
                    tab = tabp.tile([128, BL], BF16, tag=f"tab{p}")
                    nc.vector.tensor_tensor(
                        out=tab[:], in0=cl[:], in1=psr[:], op=ALU.min
                    )
                    tables.append(tab)

            # ---- rule firing ----
            firing = []
            with tc.tile_pool(name="psg", bufs=4, space="PSUM") as psgp:
                for t in range(RT):
                    fir = firep.tile([128, BL], BF16, tag=f"fir{t}")
                    for h in range(NH):
                        hsl = slice(h * HB, (h + 1) * HB)
                        gps = []
                        for g in range(G):
                            ps = psgp.tile([128, HB], F32, tag="gather")
                            p, half = g // 2, g % 2
                            base = 64 * half
                            rhs_tab = tables[p][base : base + MM2, :]
                            lhsT = gp[
                                base : base + MM2,
                                (g * RT + t) * 128 : (g * RT + t + 1) * 128,
                            ]
                            for n in range(HB // 512):
                                nc.tensor.matmul(
                                    out=ps[:, 512 * n : 512 * (n + 1)],
                                    lhsT=lhsT,
                                    rhs=rhs_tab[:, h * HB + 512 * n : h * HB + 512 * (n + 1)],
                                    start=True, stop=True,
                                )
                            gps.append(ps)
                            # drain ACT-copy groups as soon as produced
                            if g < N_ACT_COPY:
                                cg = cpp.tile([128, HB], BF16, tag=f"cg{g}")
                                nc.scalar.activation(cg[:], ps[:], AF.Copy)
                                gps[g] = cg
                        # DVE tree over the ACT-copied bf16 values (2x mode)
                        d0 = cpp.tile([128, HB], BF16, tag="d0")
                        nc.vector.tensor_tensor(
                            out=d0[:], in0=gps[0][:], in1=gps[1][:], op=ALU.min
                        )
                        d1 = cpp.tile([128, HB], BF16, tag="d1")
                        nc.vector.tensor_tensor(
                            out=d1[:], in0=gps[2][:], in1=gps[3][:], op=ALU.min
                        )
                        e = cpp.tile([128, HB], BF16, tag="e")
                        nc.vector.tensor_tensor(
                            out=e[:], in0=d0[:], in1=d1[:], op=ALU.min
                        )
                        # chain remaining groups straight from PSUM
                        run = e
                        for g in range(N_ACT_COPY, G):
                            nxt = cpp.tile([128, HB], BF16, tag="run")
                            nc.vector.tensor_tensor(
                                out=nxt[:], in0=run[:], in1=gps[g][:], op=ALU.min
                            )
                            run = nxt
                        # final relu into the firing tile
                        nc.vector.tensor_scalar(
                            out=fir[:, hsl], in0=run[:], scalar1=0.0,
                            scalar2=None, op0=ALU.max,
                        )
                    firing.append(fir)

            # ---- class segment-sum ----
            with tc.tile_pool(name="psc", bufs=1, space="PSUM") as pscp:
                psc = pscp.tile([C, BL], F32)
                for t in range(RT):
                    for n in range(BL // 512):
                        nc.tensor.matmul(
                            out=psc[:, 512 * n : 512 * (n + 1)],
                            lhsT=ch[:, t * C : (t + 1) * C],
                            rhs=firing[t][:, 512 * n : 512 * (n + 1)],
                            start=(t == 0),
                            stop=(t == RT - 1),
                        )
                outs = workp.tile([C, BL], F32)
                nc.scalar.activation(outs[:], psc[:], AF.Copy)
                nc.sync.dma_start(out_d[:], outs[:])

    _split_multi_waits(nc)
    return nc


def _host_inputs(x, mf_abc, rule_conditions, rule_classes):
    x = np.ascontiguousarray(np.asarray(x, dtype=np.float32))
    abc = np.asarray(mf_abc, dtype=np.float32).reshape(FM, 3)
    cond = np.asarray(rule_conditions).astype(np.int64)
    cls = np.asarray(rule_classes).astype(np.int64)

    a, b_, c_ = abc[:, 0], abc[:, 1], abc[:, 2]
    w1 = 1.0 / (b_ - a)
    p2 = -1.0 / (c_ - b_)
    prm = np.stack([w1, -a * w1, p2, -c_ * p2], axis=1).astype(np.float32)

    # replication one-hots for table build. Packed tile p: rows j<49 belong
    # to group 2p (features 4p, 4p+1), rows 64<=j<113 to group 2p+1
    # (features 4p+2, 4p+3). L side replicates m1=j//7, R side m2=j%7.
    rl = np.zeros([FM, NP, 128], dtype=BF16_NP)
    rr = np.zeros([FM, NP, 128], dtype=BF16_NP)
    j49 = np.arange(MM2)
    for p in range(NP):
        rl[4 * p * M + j49 // M, p, j49] = 1
        rr[(4 * p + 1) * M + j49 % M, p, j49] = 1
        rl[(4 * p + 2) * M + j49 // M, p, 64 + j49] = 1
        rr[(4 * p + 3) * M + j49 % M, p, 64 + j49] = 1
    rl = np.ascontiguousarray(rl.reshape(FM, NP * 128))
    rr = np.ascontiguousarray(rr.reshape(FM, NP * 128))

    # pair-combo gather one-hots (odd groups offset to partition base 64)
    j = np.arange(R)
    t_idx, jj = j // 128, j % 128
    gpm = np.zeros([128, G, RT, 128], dtype=BF16_NP)
    for g in range(G):
        combo = cond[:, 2 * g] * M + cond[:, 2 * g + 1] + 64 * (g % 2)
        gpm[combo, g, t_idx, jj] = 1
    gpm = np.ascontiguousarray(gpm.reshape(128, G * RT * 128))

    chm = np.zeros([128, RT, C], dtype=BF16_NP)
    chm[jj, t_idx, cls] = 1
    chm = np.ascontiguousarray(chm.reshape(128, RT * C))

    return x, prm, rl, rr, gpm, chm


def kernel(x, mf_abc, rule_conditions, rule_classes):
    global _PROGRAM
    if _PROGRAM is None:
        _PROGRAM = _build_program()

    xf, prm, rl, rr, gpm, chm = _host_inputs(
        x, mf_abc, rule_conditions, rule_classes
    )

    in_maps = [
        {
            "x": np.ascontiguousarray(xf[:, i * BL : (i + 1) * BL]),
            "prm": prm,
            "rl": rl,
            "rr": rr,
            "gp": gpm,
            "ch": chm,
        }
        for i in range(NCORES)
    ]
    res = run_bass_kernel_spmd(_PROGRAM, in_maps, core_ids=list(range(NCORES)))
    out = np.concatenate([r["out"].T for r in res.results], axis=0)
    return np.ascontiguousarray(out.astype(np.float32))


# revision 30
# speedup vs baseline: 1.7028x; 1.0912x over previous
"""NefClass fuzzy-rule classifier kernel for 8x Trainium2 NeuronCores.

Math: out[b,c] = sum_{r: class[r]=c} relu(min_f raw_mem[f, cond[r,f], b])
where raw_mem = min((x-a)/(b-a), (c-x)/(c-b)) (relu commutes with min, and
min(left,right) <= 1 always for triangular MFs, so clip reduces to one relu
applied to the final firing).

Per core (batch-sharded 8 ways, 2048 cols each):
  1. x -> x_rep [112, 2048]; raw memberships via ACT affines + DVE min (bf16).
  2. Pair tables: for each pair of features (2g, 2g+1), a 49-row table of
     min(mem_f1[m1], mem_f2[m2]) for all (m1, m2) combos. Built by PE
     replication matmuls (one-hot lhsT) + ACT copy + DVE min. Two groups are
     packed per [128, B] tile at partition bases 0 and 64 (matmul rhs base
     partition must be 0/32/64).
  3. Rule firing: for each rule tile (128 rules), gather one 49-combo row per
     pair group via one-hot PE matmul, then min-combine the 8 group values:
     ACT copies half to SBUF bf16 (DVE tree mins at 2x), DVE chains the rest
     straight from PSUM. Final relu on DVE.
  4. Class segment-sum via one-hot class matmul accumulating [10, B] in PSUM.
  5. Output [10, 2048] per core; host transposes/concats.

Rule tables and MF params are runtime inputs (host-built one-hot matrices),
so the compiled program is input-independent and cached.
"""

import numpy as np
import ml_dtypes

import concourse.bass as bass
import concourse.mybir as mybir
import concourse.tile as tile
from concourse.bass_utils import run_bass_kernel_spmd

F = 16          # features
M = 7           # membership functions per feature
C = 10          # classes
R = 512         # rules
B = 16384       # batch
NCORES = 8
BL = B // NCORES     # 2048 batch per core
FM = F * M           # 112
RT = R // 128        # 4 rule tiles of 128 rules
G = F // 2           # 8 pair groups
NP = G // 2          # 4 packed table tiles (2 groups per tile)
MM2 = M * M          # 49 combos per pair
HB = 1024            # psum chunk width in rule phase
NH = BL // HB        # 2 chunks
N_DMA = 4            # groups gathered via indirect DMA (SBUF bf16 direct)
N_PE = G - N_DMA     # groups gathered via PE one-hot matmul
N_ACT_COPY = 2       # PE groups drained via ACT copies per chunk

F32 = mybir.dt.float32
BF16 = mybir.dt.bfloat16
BF16_NP = ml_dtypes.bfloat16

AF = mybir.ActivationFunctionType
ALU = mybir.AluOpType

_PROGRAM = None


def _split_multi_waits(nc):
    """This container's walrus codegen only encodes ONE sem wait per
    instruction. Hoist extra waits into standalone NOPs on the same engine
    immediately before the instruction (same semantics: the engine's
    sequencer stalls at the NOP)."""
    k = 0
    for fn in nc.m.functions:
        for blk in fn.blocks:
            old = list(blk.instructions)
            new = []
            changed = False
            for ins in old:
                si = getattr(ins, "sync_info", None)
                eng = getattr(ins, "engine", None)
                if si is not None and len(si.on_wait) > 1 and eng is not None:
                    waits = list(si.on_wait)
                    for w in waits[:-1]:
                        nop = mybir.InstNoOp(
                            name=f"{ins.name}_ws{k}",
                            sync_info=mybir.SyncInfo(on_wait=[w], on_update=[]),
                            bass_nofuse=True,
                            engine=eng,
                        )
                        k += 1
                        new.append(nop)
                    ins.sync_info = mybir.SyncInfo(
                        on_wait=[waits[-1]], on_update=list(si.on_update)
                    )
                    changed = True
                new.append(ins)
            if changed:
                blk.instructions = new


def _build_program():
    nc = bass.Bass("TRN2", target_bir_lowering=False)

    x_d = nc.dram_tensor("x", [F, BL], F32, kind="ExternalInput").ap()
    prm_d = nc.dram_tensor("prm", [FM, 4], F32, kind="ExternalInput").ap()
    # replication one-hots: L and R side, 4 packed tiles each, [112, 128]
    rl_d = nc.dram_tensor("rl", [FM, NP * 128], BF16, kind="ExternalInput").ap()
    rr_d = nc.dram_tensor("rr", [FM, NP * 128], BF16, kind="ExternalInput").ap()
    # pair-combo gather one-hots; odd groups live at partition base 64 to
    # match their rhs table half (matmul needs equal base partitions)
    gp_d = nc.dram_tensor("gp", [128, G * RT * 128], BF16, kind="ExternalInput").ap()
    ch_d = nc.dram_tensor("ch", [128, RT * C], BF16, kind="ExternalInput").ap()
    # row index into tabd per (rule-in-tile, group, rule-tile)
    idx_d = nc.dram_tensor("idx", [128, G * RT], mybir.dt.int32,
                           kind="ExternalInput").ap()
    out_d = nc.dram_tensor("out", [C, BL], F32, kind="ExternalOutput").ap()
    # pair tables staged in DRAM for indirect-DMA row gathers
    tabd = nc.dram_tensor("tabd", [NP * 128, BL], BF16).ap()

    with tile.TileContext(nc) as tc:
        with (
            tc.tile_pool(name="const", bufs=1) as constp,
            tc.tile_pool(name="work", bufs=1) as workp,
            tc.tile_pool(name="tab", bufs=1) as tabp,
            tc.tile_pool(name="fire", bufs=1) as firep,
            tc.tile_pool(name="cpy", bufs=2) as cpp,
        ):
            prm = constp.tile([FM, 4], F32)
            nc.sync.dma_start(prm[:], prm_d[:])
            rl = constp.tile([FM, NP * 128], BF16)
            nc.sync.dma_start(rl[:], rl_d[:])
            rr = constp.tile([FM, NP * 128], BF16)
            nc.sync.dma_start(rr[:], rr_d[:])
            gp = constp.tile([128, G * RT * 128], BF16)
            nc.sync.dma_start(gp[:], gp_d[:])
            ch = constp.tile([128, RT * C], BF16)
            nc.sync.dma_start(ch[:], ch_d[:])
            idx = constp.tile([128, G * RT], mybir.dt.int32)
            nc.sync.dma_start(idx[:], idx_d[:])

            # x replicated across the 7 MF rows of each feature
            xr = workp.tile([FM, BL], F32)
            xr3 = xr[:].rearrange("(f m) b -> f m b", m=M)
            for m in range(M):
                nc.sync.dma_start(xr3[:, m, :], x_d[:, :])

            # raw memberships (relu deferred to firing)
            left = workp.tile([FM, BL], F32)
            right = workp.tile([FM, BL], F32)
            nc.scalar.activation(
                left[:], xr[:], AF.Identity, scale=prm[:, 0:1], bias=prm[:, 1:2]
            )
            nc.scalar.activation(
                right[:], xr[:], AF.Identity, scale=prm[:, 2:3], bias=prm[:, 3:4]
            )
            mem = workp.tile([FM, BL], BF16)
            nc.vector.tensor_tensor(
                out=mem[:], in0=left[:], in1=right[:], op=ALU.min
            )

            # ---- pair tables: T_p holds group 2p at partitions 0..48 and
            # group 2p+1 at partitions 64..112 ----
            tables = []
            with (
                tc.tile_pool(name="psl", bufs=1, space="PSUM") as pslp,
                tc.tile_pool(name="psr", bufs=1, space="PSUM") as psrp,
            ):
                for p in range(NP):
                    psl = pslp.tile([128, BL], F32, tag="psl")
                    psr = psrp.tile([128, BL], F32, tag="psr")
                    for n in range(BL // 512):
                        sl = slice(512 * n, 512 * (n + 1))
                        nc.tensor.matmul(
                            out=psl[:, sl], lhsT=rl[:, 128 * p : 128 * (p + 1)],
                            rhs=mem[:, sl], start=True, stop=True,
                        )
                        nc.tensor.matmul(
                            out=psr[:, sl], lhsT=rr[:, 128 * p : 128 * (p + 1)],
                            rhs=mem[:, sl], start=True, stop=True,
                        )
                    cl = workp.tile([128, BL], BF16, tag="cl")
                    nc.scalar.activation(cl[:], psl[:], AF.Copy)
                    tab = tabp.tile([128, BL], BF16, tag=f"tab{p}")
                    nc.vector.tensor_tensor(
                        out=tab[:], in0=cl[:], in1=psr[:], op=ALU.min
                    )
                    nc.sync.dma_start(tabd[128 * p : 128 * (p + 1), :], tab[:])
                    tables.append(tab)

            # ---- rule firing ----
            # groups 0..N_PE-1 gathered by PE one-hot matmul (PSUM f32);
            # groups N_PE..G-1 gathered by indirect DMA from tabd (SBUF bf16)
            firing = []
            with (
                tc.tile_pool(name="psg", bufs=4, space="PSUM") as psgp,
                tc.tile_pool(name="dmag", bufs=2) as dmagp,
            ):
                for t in range(RT):
                    fir = firep.tile([128, BL], BF16, tag=f"fir{t}")
                    dms = []
                    for g in range(N_PE, G):
                        dg = dmagp.tile([128, BL], BF16, tag=f"dg{g}")
                        col = g * RT + t
                        nc.gpsimd.indirect_dma_start(
                            out=dg[:], out_offset=None,
                            in_=tabd[:, :],
                            in_offset=bass.IndirectOffsetOnAxis(
                                ap=idx[:, col : col + 1], axis=0
                            ),
                        )
                        dms.append(dg)
                    for h in range(NH):
                        hsl = slice(h * HB, (h + 1) * HB)
                        vals = []  # (ap, is_psum)
                        for g in range(N_PE):
                            ps = psgp.tile([128, HB], F32, tag="gather")
                            p, half = g // 2, g % 2
                            base = 64 * half
                            rhs_tab = tables[p][base : base + MM2, :]
                            lhsT = gp[
                                base : base + MM2,
                                (g * RT + t) * 128 : (g * RT + t + 1) * 128,
                            ]
                            for n in range(HB // 512):
                                nc.tensor.matmul(
                                    out=ps[:, 512 * n : 512 * (n + 1)],
                                    lhsT=lhsT,
                                    rhs=rhs_tab[:, h * HB + 512 * n : h * HB + 512 * (n + 1)],
                                    start=True, stop=True,
                                )
                            if g < N_ACT_COPY:
                                cg = cpp.tile([128, HB], BF16, tag=f"cg{g}")
                                nc.scalar.activation(cg[:], ps[:], AF.Copy)
                                vals.append((cg[:], False))
                            else:
                                vals.append((ps[:], True))
                        for dg in dms:
                            vals.append((dg[:, hsl], False))
                        # pairwise-min tree over bf16 SBUF values (DVE 2x),
                        # then chain the PSUM values, then relu
                        sbuf_vals = [v for v, isp in vals if not isp]
                        psum_vals = [v for v, isp in vals if isp]
                        lvl = sbuf_vals
                        k = 0
                        while len(lvl) > 1:
                            nxt_lvl = []
                            for i in range(0, len(lvl) - 1, 2):
                                tt = cpp.tile([128, HB], BF16, tag=f"tr{k}")
                                k += 1
                                nc.vector.tensor_tensor(
                                    out=tt[:], in0=lvl[i], in1=lvl[i + 1],
                                    op=ALU.min,
                                )
                                nxt_lvl.append(tt[:])
                            if len(lvl) % 2:
                                nxt_lvl.append(lvl[-1])
                            lvl = nxt_lvl
                        run = lvl[0]
                        for pv in psum_vals:
                            nxt = cpp.tile([128, HB], BF16, tag=f"tr{k}")
                            k += 1
                            nc.vector.tensor_tensor(
                                out=nxt[:], in0=run, in1=pv, op=ALU.min
                            )
                            run = nxt[:]
                        # final relu into the firing tile
                        nc.vector.tensor_scalar(
                            out=fir[:, hsl], in0=run, scalar1=0.0,
                            scalar2=None, op0=ALU.max,
                        )
                    firing.append(fir)

            # ---- class segment-sum ----
            with tc.tile_pool(name="psc", bufs=1, space="PSUM") as pscp:
                psc = pscp.tile([C, BL], F32)
                for t in range(RT):
                    for n in range(BL // 512):
                        nc.tensor.matmul(
                            out=psc[:, 512 * n : 512 * (n + 1)],
                            lhsT=ch[:, t * C : (t + 1) * C],
                            rhs=firing[t][:, 512 * n : 512 * (n + 1)],
                            start=(t == 0),
                            stop=(t == RT - 1),
                        )
                outs = workp.tile([C, BL], F32)
                nc.scalar.activation(outs[:], psc[:], AF.Copy)
                nc.sync.dma_start(out_d[:], outs[:])

    _split_multi_waits(nc)
    return nc


def _host_inputs(x, mf_abc, rule_conditions, rule_classes):
    x = np.ascontiguousarray(np.asarray(x, dtype=np.float32))
    abc = np.asarray(mf_abc, dtype=np.float32).reshape(FM, 3)
    cond = np.asarray(rule_conditions).astype(np.int64)
    cls = np.asarray(rule_classes).astype(np.int64)

    a, b_, c_ = abc[:, 0], abc[:, 1], abc[:, 2]
    w1 = 1.0 / (b_ - a)
    p2 = -1.0 / (c_ - b_)
    prm = np.stack([w1, -a * w1, p2, -c_ * p2], axis=1).astype(np.float32)

    # replication one-hots for table build. Packed tile p: rows j<49 belong
    # to group 2p (features 4p, 4p+1), rows 64<=j<113 to group 2p+1
    # (features 4p+2, 4p+3). L side replicates m1=j//7, R side m2=j%7.
    rl = np.zeros([FM, NP, 128], dtype=BF16_NP)
    rr = np.zeros([FM, NP, 128], dtype=BF16_NP)
    j49 = np.arange(MM2)
    for p in range(NP):
        rl[4 * p * M + j49 // M, p, j49] = 1
        rr[(4 * p + 1) * M + j49 % M, p, j49] = 1
        rl[(4 * p + 2) * M + j49 // M, p, 64 + j49] = 1
        rr[(4 * p + 3) * M + j49 % M, p, 64 + j49] = 1
    rl = np.ascontiguousarray(rl.reshape(FM, NP * 128))
    rr = np.ascontiguousarray(rr.reshape(FM, NP * 128))

    # pair-combo gather one-hots (odd groups offset to partition base 64)
    j = np.arange(R)
    t_idx, jj = j // 128, j % 128
    gpm = np.zeros([128, G, RT, 128], dtype=BF16_NP)
    for g in range(G):
        combo = cond[:, 2 * g] * M + cond[:, 2 * g + 1] + 64 * (g % 2)
        gpm[combo, g, t_idx, jj] = 1
    gpm = np.ascontiguousarray(gpm.reshape(128, G * RT * 128))

    chm = np.zeros([128, RT, C], dtype=BF16_NP)
    chm[jj, t_idx, cls] = 1
    chm = np.ascontiguousarray(chm.reshape(128, RT * C))

    # tabd row index per (rule-in-tile, group, rule-tile)
    idx = np.zeros([128, G, RT], dtype=np.int32)
    for g in range(G):
        combo = cond[:, 2 * g] * M + cond[:, 2 * g + 1]
        idx[jj, g, t_idx] = (g // 2) * 128 + 64 * (g % 2) + combo
    idx = np.ascontiguousarray(idx.reshape(128, G * RT))

    return x, prm, rl, rr, gpm, chm, idx


def kernel(x, mf_abc, rule_conditions, rule_classes):
    global _PROGRAM
    if _PROGRAM is None:
        _PROGRAM = _build_program()

    xf, prm, rl, rr, gpm, chm, idx = _host_inputs(
        x, mf_abc, rule_conditions, rule_classes
    )

    in_maps = [
        {
            "x": np.ascontiguousarray(xf[:, i * BL : (i + 1) * BL]),
            "prm": prm,
            "rl": rl,
            "rr": rr,
            "gp": gpm,
            "ch": chm,
            "idx": idx,
        }
        for i in range(NCORES)
    ]
    res = run_bass_kernel_spmd(_PROGRAM, in_maps, core_ids=list(range(NCORES)))
    out = np.concatenate([r["out"].T for r in res.results], axis=0)
    return np.ascontiguousarray(out.astype(np.float32))


# revision 39
# speedup vs baseline: 1.7140x; 1.0066x over previous
"""NefClass fuzzy-rule classifier kernel for 8x Trainium2 NeuronCores.

Math: out[b,c] = sum_{r: class[r]=c} relu(min_f raw_mem[f, cond[r,f], b])
where raw_mem = min((x-a)/(b-a), (c-x)/(c-b)) (relu commutes with min, and
min(left,right) <= 1 always for triangular MFs, so clip reduces to one relu
applied to the final firing).

Per core (batch-sharded 8 ways, 2048 cols each):
  1. x -> x_rep [112, 2048]; raw memberships via ACT affines + DVE min (bf16).
  2. Pair tables: for each pair of features (2g, 2g+1), a 49-row table of
     min(mem_f1[m1], mem_f2[m2]) for all (m1, m2) combos. Built by PE
     replication matmuls (one-hot lhsT) + ACT copy + DVE min. Two groups are
     packed per [128, B] tile at partition bases 0 and 64 (matmul rhs base
     partition must be 0/32/64).
  3. Rule firing: for each rule tile (128 rules), gather one 49-combo row per
     pair group via one-hot PE matmul, then min-combine the 8 group values:
     ACT copies half to SBUF bf16 (DVE tree mins at 2x), DVE chains the rest
     straight from PSUM. Final relu on DVE.
  4. Class segment-sum via one-hot class matmul accumulating [10, B] in PSUM.
  5. Output [10, 2048] per core; host transposes/concats.

Rule tables and MF params are runtime inputs (host-built one-hot matrices),
so the compiled program is input-independent and cached.
"""

import numpy as np
import ml_dtypes

import concourse.bass as bass
import concourse.mybir as mybir
import concourse.tile as tile
from concourse.bass_utils import run_bass_kernel_spmd

F = 16          # features
M = 7           # membership functions per feature
C = 10          # classes
R = 512         # rules
B = 16384       # batch
NCORES = 8
BL = B // NCORES     # 2048 batch per core
FM = F * M           # 112
RT = R // 128        # 4 rule tiles of 128 rules
G = F // 2           # 8 pair groups
NP = G // 2          # 4 packed table tiles (2 groups per tile)
MM2 = M * M          # 49 combos per pair
HB = 1024            # psum chunk width (gathers + table build)
NH = BL // HB        # 2 chunks
N_DMA = 6            # groups gathered via indirect DMA (SBUF bf16 direct)
N_PE = G - N_DMA     # groups gathered via PE one-hot matmul (ACT-drained)

F32 = mybir.dt.float32
BF16 = mybir.dt.bfloat16
BF16_NP = ml_dtypes.bfloat16

AF = mybir.ActivationFunctionType
ALU = mybir.AluOpType

_PROGRAM = None


def _split_multi_waits(nc):
    """This container's walrus codegen only encodes ONE sem wait per
    instruction. Hoist extra waits into standalone NOPs on the same engine
    immediately before the instruction (same semantics: the engine's
    sequencer stalls at the NOP)."""
    k = 0
    for fn in nc.m.functions:
        for blk in fn.blocks:
            old = list(blk.instructions)
            new = []
            changed = False
            for ins in old:
                si = getattr(ins, "sync_info", None)
                eng = getattr(ins, "engine", None)
                if si is not None and len(si.on_wait) > 1 and eng is not None:
                    waits = list(si.on_wait)
                    for w in waits[:-1]:
                        nop = mybir.InstNoOp(
                            name=f"{ins.name}_ws{k}",
                            sync_info=mybir.SyncInfo(on_wait=[w], on_update=[]),
                            bass_nofuse=True,
                            engine=eng,
                        )
                        k += 1
                        new.append(nop)
                    ins.sync_info = mybir.SyncInfo(
                        on_wait=[waits[-1]], on_update=list(si.on_update)
                    )
                    changed = True
                new.append(ins)
            if changed:
                blk.instructions = new


def _build_program():
    nc = bass.Bass("TRN2", target_bir_lowering=False)

    x_d = nc.dram_tensor("x", [F, BL], F32, kind="ExternalInput").ap()
    prm_d = nc.dram_tensor("prm", [FM, 4], F32, kind="ExternalInput").ap()
    # replication one-hots: L and R side, 4 packed tiles each, [112, 128]
    rl_d = nc.dram_tensor("rl", [FM, NP * 128], BF16, kind="ExternalInput").ap()
    rr_d = nc.dram_tensor("rr", [FM, NP * 128], BF16, kind="ExternalInput").ap()
    # pair-combo gather one-hots for the PE-gathered groups; odd groups live
    # at partition base 64 to match their rhs table half
    gp_d = nc.dram_tensor("gp", [128, N_PE * RT * 128], BF16,
                          kind="ExternalInput").ap()
    ch_d = nc.dram_tensor("ch", [128, RT * C], BF16, kind="ExternalInput").ap()
    # row index into tabd per (rule-in-tile, dma-group, rule-tile)
    idx_d = nc.dram_tensor("idx", [128, N_DMA * RT], mybir.dt.int32,
                           kind="ExternalInput").ap()
    out_d = nc.dram_tensor("out", [C, BL], F32, kind="ExternalOutput").ap()
    # pair tables staged in DRAM for indirect-DMA row gathers
    tabd = nc.dram_tensor("tabd", [NP * 128, BL], BF16).ap()

    with tile.TileContext(nc) as tc:
        with (
            tc.tile_pool(name="const", bufs=1) as constp,
            tc.tile_pool(name="work", bufs=1) as workp,
            tc.tile_pool(name="tab", bufs=1) as tabp,
            tc.tile_pool(name="fire", bufs=1) as firep,
            tc.tile_pool(name="cpy", bufs=2) as cpp,
            tc.tile_pool(name="tree", bufs=6) as trp,
            tc.tile_pool(name="dmag", bufs=3) as dmagp,
        ):
            # compute-critical inputs first (DMAs on one HWDGE ring are FIFO)
            prm = constp.tile([FM, 4], F32)
            nc.sync.dma_start(prm[:], prm_d[:])
            xr = workp.tile([FM, BL], F32)
            xr3 = xr[:].rearrange("(f m) b -> f m b", m=M)
            for m in range(M):
                nc.sync.dma_start(xr3[:, m, :], x_d[:, :])
            rl = constp.tile([FM, NP * 128], BF16)
            nc.sync.dma_start(rl[:], rl_d[:])
            rr = constp.tile([FM, NP * 128], BF16)
            nc.sync.dma_start(rr[:], rr_d[:])
            gp = constp.tile([128, N_PE * RT * 128], BF16)
            nc.sync.dma_start(gp[:], gp_d[:])
            idx = constp.tile([128, N_DMA * RT], mybir.dt.int32)
            nc.sync.dma_start(idx[:], idx_d[:])
            ch = constp.tile([128, RT * C], BF16)
            nc.sync.dma_start(ch[:], ch_d[:])

            # raw memberships (relu deferred to firing)
            left = workp.tile([FM, BL], F32)
            nc.scalar.activation(
                left[:], xr[:], AF.Identity, scale=prm[:, 0:1], bias=prm[:, 1:2]
            )
            nc.scalar.activation(
                xr[:], xr[:], AF.Identity, scale=prm[:, 2:3], bias=prm[:, 3:4]
            )
            mem = workp.tile([FM, BL], BF16)
            nc.vector.tensor_tensor(
                out=mem[:], in0=left[:], in1=xr[:], op=ALU.min
            )

            # ---- pair tables (interleaved with rule gathers on PE) ----
            # T_p holds group 2p at partitions 0..48, group 2p+1 at 64..112.
            # PE groups are 0..N_PE-1 (table 0), built first so PE gather
            # matmuls can stream right behind the table matmuls.
            firing = []
            for t in range(RT):
                fir = firep.tile([128, BL], BF16, tag=f"fir{t}")
                firing.append(fir)
            tvals = [[] for _ in range(RT)]  # per rule tile: bf16 SBUF values
            tables = []
            with (
                tc.tile_pool(name="psl", bufs=1, space="PSUM") as pslp,
                tc.tile_pool(name="psr", bufs=1, space="PSUM") as psrp,
                tc.tile_pool(name="psg", bufs=2, space="PSUM") as psgp,
            ):
                for p in range(NP):
                    tab = tabp.tile([128, BL], BF16, tag=f"tab{p}")
                    for n in range(NH):
                        sl = slice(HB * n, HB * (n + 1))
                        psl = pslp.tile([128, HB], F32, tag="psl")
                        psr = psrp.tile([128, HB], F32, tag="psr")
                        for q in range(HB // 512):
                            qsl = slice(512 * q, 512 * (q + 1))
                            msl = slice(HB * n + 512 * q, HB * n + 512 * (q + 1))
                            nc.tensor.matmul(
                                out=psl[:, qsl],
                                lhsT=rl[:, 128 * p : 128 * (p + 1)],
                                rhs=mem[:, msl], start=True, stop=True,
                            )
                            nc.tensor.matmul(
                                out=psr[:, qsl],
                                lhsT=rr[:, 128 * p : 128 * (p + 1)],
                                rhs=mem[:, msl], start=True, stop=True,
                            )
                        cl = workp.tile([128, HB], BF16, tag="cl")
                        nc.scalar.activation(cl[:], psl[:], AF.Copy)
                        nc.vector.tensor_tensor(
                            out=tab[:, sl], in0=cl[:], in1=psr[:], op=ALU.min
                        )
                    tables.append(tab)
                    nc.sync.dma_start(tabd[128 * p : 128 * (p + 1), :], tab[:])

                # per rule tile: gathers (DMA + PE) then min tree + relu
                for t in range(RT):
                    for g in range(N_PE, G):
                        dg = dmagp.tile([128, BL], BF16, tag=f"dg{g}")
                        col = (g - N_PE) * RT + t
                        nc.gpsimd.indirect_dma_start(
                            out=dg[:], out_offset=None,
                            in_=tabd[:, :],
                            in_offset=bass.IndirectOffsetOnAxis(
                                ap=idx[:, col : col + 1], axis=0
                            ),
                        )
                        tvals[t].append(dg[:])
                    for g in range(N_PE):
                        base = 64 * (g % 2)
                        rhs_tab = tables[g // 2][base : base + MM2, :]
                        lhsT = gp[
                            base : base + MM2,
                            (g * RT + t) * 128 : (g * RT + t + 1) * 128,
                        ]
                        cg = cpp.tile([128, BL], BF16, tag=f"cg{g}")
                        for n in range(NH):
                            ps = psgp.tile([128, HB], F32, tag="gather")
                            for q in range(HB // 512):
                                nc.tensor.matmul(
                                    out=ps[:, 512 * q : 512 * (q + 1)],
                                    lhsT=lhsT,
                                    rhs=rhs_tab[:, HB * n + 512 * q : HB * n + 512 * (q + 1)],
                                    start=True, stop=True,
                                )
                            nc.scalar.activation(
                                cg[:, HB * n : HB * (n + 1)], ps[:], AF.Copy
                            )
                        tvals[t].append(cg[:])

                    # min tree (all bf16 SBUF, DVE 2x mode) + relu
                    lvl = tvals[t]
                    while len(lvl) > 1:
                        nxt_lvl = []
                        for i in range(0, len(lvl) - 1, 2):
                            tt = trp.tile([128, BL], BF16, tag="tr")
                            nc.vector.tensor_tensor(
                                out=tt[:], in0=lvl[i], in1=lvl[i + 1], op=ALU.min
                            )
                            nxt_lvl.append(tt[:])
                        if len(lvl) % 2:
                            nxt_lvl.append(lvl[-1])
                        lvl = nxt_lvl
                    nc.vector.tensor_scalar(
                        out=firing[t][:], in0=lvl[0], scalar1=0.0,
                        scalar2=None, op0=ALU.max,
                    )

            # ---- class segment-sum ----
            with tc.tile_pool(name="psc", bufs=1, space="PSUM") as pscp:
                psc = pscp.tile([C, BL], F32)
                for t in range(RT):
                    for n in range(BL // 512):
                        nc.tensor.matmul(
                            out=psc[:, 512 * n : 512 * (n + 1)],
                            lhsT=ch[:, t * C : (t + 1) * C],
                            rhs=firing[t][:, 512 * n : 512 * (n + 1)],
                            start=(t == 0),
                            stop=(t == RT - 1),
                        )
                outs = workp.tile([C, BL], F32)
                nc.scalar.activation(outs[:], psc[:], AF.Copy)
                nc.sync.dma_start(out_d[:], outs[:])

    _split_multi_waits(nc)
    return nc


def _host_inputs(x, mf_abc, rule_conditions, rule_classes):
    x = np.ascontiguousarray(np.asarray(x, dtype=np.float32))
    abc = np.asarray(mf_abc, dtype=np.float32).reshape(FM, 3)
    cond = np.asarray(rule_conditions).astype(np.int64)
    cls = np.asarray(rule_classes).astype(np.int64)

    a, b_, c_ = abc[:, 0], abc[:, 1], abc[:, 2]
    w1 = 1.0 / (b_ - a)
    p2 = -1.0 / (c_ - b_)
    prm = np.stack([w1, -a * w1, p2, -c_ * p2], axis=1).astype(np.float32)

    # replication one-hots for table build. Packed tile p: rows j<49 belong
    # to group 2p (features 4p, 4p+1), rows 64<=j<113 to group 2p+1
    # (features 4p+2, 4p+3). L side replicates m1=j//7, R side m2=j%7.
    rl = np.zeros([FM, NP, 128], dtype=BF16_NP)
    rr = np.zeros([FM, NP, 128], dtype=BF16_NP)
    j49 = np.arange(MM2)
    for p in range(NP):
        rl[4 * p * M + j49 // M, p, j49] = 1
        rr[(4 * p + 1) * M + j49 % M, p, j49] = 1
        rl[(4 * p + 2) * M + j49 // M, p, 64 + j49] = 1
        rr[(4 * p + 3) * M + j49 % M, p, 64 + j49] = 1
    rl = np.ascontiguousarray(rl.reshape(FM, NP * 128))
    rr = np.ascontiguousarray(rr.reshape(FM, NP * 128))

    # pair-combo gather one-hots for PE groups (odd groups at base 64)
    j = np.arange(R)
    t_idx, jj = j // 128, j % 128
    gpm = np.zeros([128, N_PE, RT, 128], dtype=BF16_NP)
    for g in range(N_PE):
        combo = cond[:, 2 * g] * M + cond[:, 2 * g + 1] + 64 * (g % 2)
        gpm[combo, g, t_idx, jj] = 1
    gpm = np.ascontiguousarray(gpm.reshape(128, N_PE * RT * 128))

    chm = np.zeros([128, RT, C], dtype=BF16_NP)
    chm[jj, t_idx, cls] = 1
    chm = np.ascontiguousarray(chm.reshape(128, RT * C))

    # tabd row index per (rule-in-tile, dma-group, rule-tile)
    idx = np.zeros([128, N_DMA, RT], dtype=np.int32)
    for g in range(N_PE, G):
        combo = cond[:, 2 * g] * M + cond[:, 2 * g + 1]
        idx[jj, g - N_PE, t_idx] = (g // 2) * 128 + 64 * (g % 2) + combo
    idx = np.ascontiguousarray(idx.reshape(128, N_DMA * RT))

    return x, prm, rl, rr, gpm, chm, idx


def kernel(x, mf_abc, rule_conditions, rule_classes):
    global _PROGRAM
    if _PROGRAM is None:
        _PROGRAM = _build_program()

    xf, prm, rl, rr, gpm, chm, idx = _host_inputs(
        x, mf_abc, rule_conditions, rule_classes
    )

    in_maps = [
        {
            "x": np.ascontiguousarray(xf[:, i * BL : (i + 1) * BL]),
            "prm": prm,
            "rl": rl,
            "rr": rr,
            "gp": gpm,
            "ch": chm,
            "idx": idx,
        }
        for i in range(NCORES)
    ]
    res = run_bass_kernel_spmd(_PROGRAM, in_maps, core_ids=list(range(NCORES)))
    out = np.concatenate([r["out"].T for r in res.results], axis=0)
    return np.ascontiguousarray(out.astype(np.float32))


# revision 43
# speedup vs baseline: 1.8351x; 1.0707x over previous
"""NefClass fuzzy-rule classifier kernel for 8x Trainium2 NeuronCores.

Math: out[b,c] = sum_{r: class[r]=c} relu(min_f raw_mem[f, cond[r,f], b])
where raw_mem = min((x-a)/(b-a), (c-x)/(c-b)) (relu commutes with min, and
min(left,right) <= 1 always for triangular MFs, so clip reduces to one relu
applied to the final firing).

Per core (batch-sharded 8 ways, 2048 cols each):
  1. x -> x_rep [112, 2048]; raw memberships via ACT affines + DVE min (bf16).
  2. Pair tables: for each pair of features (2g, 2g+1), a 49-row table of
     min(mem_f1[m1], mem_f2[m2]) for all (m1, m2) combos. Built by PE
     replication matmuls (one-hot lhsT) + ACT copy + DVE min. Two groups are
     packed per [128, B] tile at partition bases 0 and 64 (matmul rhs base
     partition must be 0/32/64).
  3. Rule firing: for each rule tile (128 rules), gather one 49-combo row per
     pair group via one-hot PE matmul, then min-combine the 8 group values:
     ACT copies half to SBUF bf16 (DVE tree mins at 2x), DVE chains the rest
     straight from PSUM. Final relu on DVE.
  4. Class segment-sum via one-hot class matmul accumulating [10, B] in PSUM.
  5. Output [10, 2048] per core; host transposes/concats.

Rule tables and MF params are runtime inputs (host-built one-hot matrices),
so the compiled program is input-independent and cached.
"""

import numpy as np
import ml_dtypes

import concourse.bass as bass
import concourse.mybir as mybir
import concourse.tile as tile
from concourse.bass_utils import run_bass_kernel_spmd

F = 16          # features
M = 7           # membership functions per feature
C = 10          # classes
R = 512         # rules
B = 16384       # batch
NCORES = 8
BL = B // NCORES     # 2048 batch per core
FM = F * M           # 112
RT = R // 128        # 4 rule tiles of 128 rules
G = F // 2           # 8 pair groups
NP = G // 2          # 4 packed table tiles (2 groups per tile)
MM2 = M * M          # 49 combos per pair
HB = 1024            # psum chunk width (gathers + table build)
NH = BL // HB        # 2 chunks
N_DMA = 6            # groups gathered via indirect DMA (SBUF bf16 direct)
N_PE = G - N_DMA     # groups gathered via PE one-hot matmul (ACT-drained)

F32 = mybir.dt.float32
BF16 = mybir.dt.bfloat16
BF16_NP = ml_dtypes.bfloat16

AF = mybir.ActivationFunctionType
ALU = mybir.AluOpType

_PROGRAM = None


def _split_multi_waits(nc):
    """This container's walrus codegen only encodes ONE sem wait per
    instruction. Hoist extra waits into standalone NOPs on the same engine
    immediately before the instruction (same semantics: the engine's
    sequencer stalls at the NOP)."""
    k = 0
    for fn in nc.m.functions:
        for blk in fn.blocks:
            old = list(blk.instructions)
            new = []
            changed = False
            for ins in old:
                si = getattr(ins, "sync_info", None)
                eng = getattr(ins, "engine", None)
                if si is not None and len(si.on_wait) > 1 and eng is not None:
                    waits = list(si.on_wait)
                    for w in waits[:-1]:
                        nop = mybir.InstNoOp(
                            name=f"{ins.name}_ws{k}",
                            sync_info=mybir.SyncInfo(on_wait=[w], on_update=[]),
                            bass_nofuse=True,
                            engine=eng,
                        )
                        k += 1
                        new.append(nop)
                    ins.sync_info = mybir.SyncInfo(
                        on_wait=[waits[-1]], on_update=list(si.on_update)
                    )
                    changed = True
                new.append(ins)
            if changed:
                blk.instructions = new


def _build_program():
    nc = bass.Bass("TRN2", target_bir_lowering=False)

    x_d = nc.dram_tensor("x", [F, BL], F32, kind="ExternalInput").ap()
    prm_d = nc.dram_tensor("prm", [FM, 4], F32, kind="ExternalInput").ap()
    # replication one-hots: L and R side, 4 packed tiles each, [112, 128]
    rl_d = nc.dram_tensor("rl", [FM, NP * 128], BF16, kind="ExternalInput").ap()
    rr_d = nc.dram_tensor("rr", [FM, NP * 128], BF16, kind="ExternalInput").ap()
    # pair-combo gather one-hots for the PE-gathered groups; odd groups live
    # at partition base 64 to match their rhs table half
    gp_d = nc.dram_tensor("gp", [128, N_PE * RT * 128], BF16,
                          kind="ExternalInput").ap()
    ch_d = nc.dram_tensor("ch", [128, RT * C], BF16, kind="ExternalInput").ap()
    # row index into tabd per (rule-in-tile, dma-group, rule-tile)
    idx_d = nc.dram_tensor("idx", [128, N_DMA * RT], mybir.dt.int32,
                           kind="ExternalInput").ap()
    out_d = nc.dram_tensor("out", [C, BL], F32, kind="ExternalOutput").ap()
    # pair tables staged in DRAM for indirect-DMA row gathers; one tensor
    # per table so each gather depends only on its own table's write
    tabds = [
        nc.dram_tensor(f"tabd{p}", [128, BL], BF16).ap() for p in range(NP)
    ]

    with tile.TileContext(nc) as tc:
        with (
            tc.tile_pool(name="const", bufs=1) as constp,
            tc.tile_pool(name="work", bufs=1) as workp,
            tc.tile_pool(name="tab", bufs=1) as tabp,
            tc.tile_pool(name="fire", bufs=1) as firep,
            tc.tile_pool(name="cpy", bufs=2) as cpp,
            tc.tile_pool(name="tree", bufs=6) as trp,
            tc.tile_pool(name="dmag", bufs=3) as dmagp,
        ):
            # compute-critical inputs first (DMAs on one HWDGE ring are FIFO)
            prm = constp.tile([FM, 4], F32)
            nc.sync.dma_start(prm[:], prm_d[:])
            # x replication on the ACT HWDGE ring, in parallel with the
            # constant loads on the SP ring
            xr = workp.tile([FM, BL], F32)
            xr3 = xr[:].rearrange("(f m) b -> f m b", m=M)
            for m in range(M):
                nc.scalar.dma_start(xr3[:, m, :], x_d[:, :])
            rl = constp.tile([FM, NP * 128], BF16)
            nc.sync.dma_start(rl[:], rl_d[:])
            rr = constp.tile([FM, NP * 128], BF16)
            nc.sync.dma_start(rr[:], rr_d[:])
            gp = constp.tile([128, N_PE * RT * 128], BF16)
            nc.sync.dma_start(gp[:], gp_d[:])
            idx = constp.tile([128, N_DMA * RT], mybir.dt.int32)
            nc.sync.dma_start(idx[:], idx_d[:])
            ch = constp.tile([128, RT * C], BF16)
            nc.sync.dma_start(ch[:], ch_d[:])

            # raw memberships (relu deferred to firing)
            left = workp.tile([FM, BL], F32)
            nc.scalar.activation(
                left[:], xr[:], AF.Identity, scale=prm[:, 0:1], bias=prm[:, 1:2]
            )
            nc.scalar.activation(
                xr[:], xr[:], AF.Identity, scale=prm[:, 2:3], bias=prm[:, 3:4]
            )
            mem = workp.tile([FM, BL], BF16)
            nc.vector.tensor_tensor(
                out=mem[:], in0=left[:], in1=xr[:], op=ALU.min
            )

            # ---- pair tables (interleaved with rule gathers on PE) ----
            # T_p holds group 2p at partitions 0..48, group 2p+1 at 64..112.
            # PE groups are 0..N_PE-1 (table 0), built first so PE gather
            # matmuls can stream right behind the table matmuls.
            firing = []
            for t in range(RT):
                fir = firep.tile([128, BL], BF16, tag=f"fir{t}")
                firing.append(fir)
            tvals = [[] for _ in range(RT)]  # per rule tile: bf16 SBUF values
            tables = []
            outs = workp.tile([C, BL], F32)
            with (
                tc.tile_pool(name="psl", bufs=2, space="PSUM") as pslp,
                tc.tile_pool(name="psr", bufs=2, space="PSUM") as psrp,
                tc.tile_pool(name="psg", bufs=2, space="PSUM") as psgp,
                tc.tile_pool(name="psc", bufs=1, space="PSUM") as pscp,
            ):
                for p in range(NP):
                    tab = tabp.tile([128, BL], BF16, tag=f"tab{p}")
                    for n in range(BL // 512):
                        sl = slice(512 * n, 512 * (n + 1))
                        psl = pslp.tile([128, 512], F32, tag="psl")
                        psr = psrp.tile([128, 512], F32, tag="psr")
                        nc.tensor.matmul(
                            out=psl[:, :], lhsT=rl[:, 128 * p : 128 * (p + 1)],
                            rhs=mem[:, sl], start=True, stop=True,
                        )
                        nc.tensor.matmul(
                            out=psr[:, :], lhsT=rr[:, 128 * p : 128 * (p + 1)],
                            rhs=mem[:, sl], start=True, stop=True,
                        )
                        cl = cpp.tile([128, 512], BF16, tag="cl")
                        nc.scalar.activation(cl[:], psl[:], AF.Copy)
                        nc.vector.tensor_tensor(
                            out=tab[:, sl], in0=cl[:], in1=psr[:], op=ALU.min
                        )
                    tables.append(tab)
                    nc.sync.dma_start(tabds[p][:, :], tab[:])

                # per rule tile: gathers (DMA + PE) then min tree + relu
                for t in range(RT):
                    for g in range(N_PE, G):
                        dg = dmagp.tile([128, BL], BF16, tag=f"dg{g}")
                        col = (g - N_PE) * RT + t
                        nc.gpsimd.indirect_dma_start(
                            out=dg[:], out_offset=None,
                            in_=tabds[g // 2][:, :],
                            in_offset=bass.IndirectOffsetOnAxis(
                                ap=idx[:, col : col + 1], axis=0
                            ),
                        )
                        tvals[t].append(dg[:])
                    for g in range(N_PE):
                        base = 64 * (g % 2)
                        rhs_tab = tables[g // 2][base : base + MM2, :]
                        lhsT = gp[
                            base : base + MM2,
                            (g * RT + t) * 128 : (g * RT + t + 1) * 128,
                        ]
                        cg = cpp.tile([128, BL], BF16, tag=f"cg{g}")
                        for n in range(BL // 512):
                            ps = psgp.tile([128, 512], F32, tag="gather")
                            nc.tensor.matmul(
                                out=ps[:, :], lhsT=lhsT,
                                rhs=rhs_tab[:, 512 * n : 512 * (n + 1)],
                                start=True, stop=True,
                            )
                            nc.scalar.activation(
                                cg[:, 512 * n : 512 * (n + 1)], ps[:], AF.Copy
                            )
                        tvals[t].append(cg[:])

                    # min tree (all bf16 SBUF, DVE 2x mode) + relu
                    lvl = tvals[t]
                    while len(lvl) > 1:
                        nxt_lvl = []
                        for i in range(0, len(lvl) - 1, 2):
                            tt = trp.tile([128, BL], BF16, tag="tr")
                            nc.vector.tensor_tensor(
                                out=tt[:], in0=lvl[i], in1=lvl[i + 1], op=ALU.min
                            )
                            nxt_lvl.append(tt[:])
                        if len(lvl) % 2:
                            nxt_lvl.append(lvl[-1])
                        lvl = nxt_lvl
                    nc.vector.tensor_scalar(
                        out=firing[t][:], in0=lvl[0], scalar1=0.0,
                        scalar2=None, op0=ALU.max,
                    )

                # ---- class segment-sum (two sequential 1024-wide halves) ----
                for h in range(2):
                    psc = pscp.tile([C, 1024], F32, tag="psc")
                    for t in range(RT):
                        for q in range(2):
                            off = 1024 * h + 512 * q
                            nc.tensor.matmul(
                                out=psc[:, 512 * q : 512 * (q + 1)],
                                lhsT=ch[:, t * C : (t + 1) * C],
                                rhs=firing[t][:, off : off + 512],
                                start=(t == 0),
                                stop=(t == RT - 1),
                            )
                    nc.scalar.activation(
                        outs[:, 1024 * h : 1024 * (h + 1)], psc[:], AF.Copy
                    )
                nc.sync.dma_start(out_d[:], outs[:])

    _split_multi_waits(nc)
    return nc


def _host_inputs(x, mf_abc, rule_conditions, rule_classes):
    x = np.ascontiguousarray(np.asarray(x, dtype=np.float32))
    abc = np.asarray(mf_abc, dtype=np.float32).reshape(FM, 3)
    cond = np.asarray(rule_conditions).astype(np.int64)
    cls = np.asarray(rule_classes).astype(np.int64)

    a, b_, c_ = abc[:, 0], abc[:, 1], abc[:, 2]
    w1 = 1.0 / (b_ - a)
    p2 = -1.0 / (c_ - b_)
    prm = np.stack([w1, -a * w1, p2, -c_ * p2], axis=1).astype(np.float32)

    # replication one-hots for table build. Packed tile p: rows j<49 belong
    # to group 2p (features 4p, 4p+1), rows 64<=j<113 to group 2p+1
    # (features 4p+2, 4p+3). L side replicates m1=j//7, R side m2=j%7.
    rl = np.zeros([FM, NP, 128], dtype=BF16_NP)
    rr = np.zeros([FM, NP, 128], dtype=BF16_NP)
    j49 = np.arange(MM2)
    for p in range(NP):
        rl[4 * p * M + j49 // M, p, j49] = 1
        rr[(4 * p + 1) * M + j49 % M, p, j49] = 1
        rl[(4 * p + 2) * M + j49 // M, p, 64 + j49] = 1
        rr[(4 * p + 3) * M + j49 % M, p, 64 + j49] = 1
    rl = np.ascontiguousarray(rl.reshape(FM, NP * 128))
    rr = np.ascontiguousarray(rr.reshape(FM, NP * 128))

    # pair-combo gather one-hots for PE groups (odd groups at base 64)
    j = np.arange(R)
    t_idx, jj = j // 128, j % 128
    gpm = np.zeros([128, N_PE, RT, 128], dtype=BF16_NP)
    for g in range(N_PE):
        combo = cond[:, 2 * g] * M + cond[:, 2 * g + 1] + 64 * (g % 2)
        gpm[combo, g, t_idx, jj] = 1
    gpm = np.ascontiguousarray(gpm.reshape(128, N_PE * RT * 128))

    chm = np.zeros([128, RT, C], dtype=BF16_NP)
    chm[jj, t_idx, cls] = 1
    chm = np.ascontiguousarray(chm.reshape(128, RT * C))

    # per-table row index (each dma group reads its own tabd tensor)
    idx = np.zeros([128, N_DMA, RT], dtype=np.int32)
    for g in range(N_PE, G):
        combo = cond[:, 2 * g] * M + cond[:, 2 * g + 1]
        idx[jj, g - N_PE, t_idx] = 64 * (g % 2) + combo
    idx = np.ascontiguousarray(idx.reshape(128, N_DMA * RT))

    return x, prm, rl, rr, gpm, chm, idx


def kernel(x, mf_abc, rule_conditions, rule_classes):
    global _PROGRAM
    if _PROGRAM is None:
        _PROGRAM = _build_program()

    xf, prm, rl, rr, gpm, chm, idx = _host_inputs(
        x, mf_abc, rule_conditions, rule_classes
    )

    in_maps = [
        {
            "x": np.ascontiguousarray(xf[:, i * BL : (i + 1) * BL]),
            "prm": prm,
            "rl": rl,
            "rr": rr,
            "gp": gpm,
            "ch": chm,
            "idx": idx,
        }
        for i in range(NCORES)
    ]
    res = run_bass_kernel_spmd(_PROGRAM, in_maps, core_ids=list(range(NCORES)))
    out = np.concatenate([r["out"].T for r in res.results], axis=0)
    return np.ascontiguousarray(out.astype(np.float32))
